# revision 11
# baseline (speedup 1.0000x reference)
"""Trainium2 Bass kernel for nn_CAMLocalHead (CAM target + conv head + BCE).

v3: conv head on device at fp8-DoubleRow rate; cheap parts (argmax, CAM
row matvec, min-max norm, top-k scatter, BCE reduce) on host.

Sharding: one sample per core (8 cores). Per core:
  - Conv3d(2048->512, 1x3x3, pad 011) as 9 shifted fp8 DoubleRow matmuls
    per (d-tile, c-pair) accumulating in PSUM. x stored as 3 w-shifted
    UNPADDED copies of widths (7,6,6): edge taps are trimmed to their
    valid output region (strided psum out APs) so no zero row or column
    is ever streamed.
  - Weights pre-scaled x64 into e4m3 range. Bias+ReLU fused into ONE
    DVE scalar_tensor_tensor per (d-tile, t-half): max(psum + 64*b, 0)
    -> bf16 (the 1/64 un-scale is folded into the score weights on
    host). Scalar/ACT engine is not used at all.
  - Score conv = one bf16 matmul per (d-tile, t-half) into [1, 392]
    psums accumulated across d-tiles. Score matmuls are emitted one
    d-tile late so the PE never stalls waiting on the relu.
  - Last d-tile runs u-outer (all c-pairs for t-half 0, then t-half 1)
    from a resident 8-chunk weight pool, so relu+score of half 0
    overlap the half-1 convolution and the tail shrinks.
  - Output: raw logits [1, 784] (score_b added on host). Epilogue:
    psum -> SBUF copies on DVE + GpSimd in parallel, one DMA out.

Host: cam = proj_weight[argmax(pred)] @ x (exact f32), top-392 mask and
scatter, then BCE mean over all samples in f64.
"""
import sys

for _p in ("/opt/trn_rl_repo", "/opt/pypackages"):
    if _p not in sys.path:
        sys.path.append(_p)

import numpy as np
import ml_dtypes

# Problem dims (hardcoded per spec)
B, C, T, H, W = 8, 2048, 16, 7, 7
K, D = 400, 512
N_TOKEN = 392
P = 128
CT = C // P          # 16 c-tiles
CTP = CT // 2        # 8 c-tile pairs (DoubleRow)
DT = D // P          # 4 d-tiles
NH = 2               # t halves (t 0..7, 8..15)
TH = T // NH         # 8
NF = TH * H * W      # 392 positions per half
NPOS = T * H * W     # 784
CW = (7, 6, 6)       # copy widths for dw = 1, 0, 2 (stored s1, s0, s2)
SPT1 = T * 7 * CW[0]  # 784:  s=1 copy (raw x), all t
SPT0 = T * 7 * CW[1]  # 672:  s=0 copy, all t
SOFF = {1: 0, 0: SPT1, 2: SPT1 + SPT0}   # offsets within one c-plane
CPL = SPT1 + 2 * SPT0  # 2128: one c-plane's three copies
XF = 2 * CPL         # 4256: free size of one fp8 x pair-tile
WSCALE = 64.0

# taps ordered so ctp0 starts with the full-coverage center tap (its
# start=True matmul initializes the whole psum region) and the s=1 copy
# is needed first, then s=0, then s=2 (matching the x DMA split order).
TAPS = [(1, 1), (0, 1), (2, 1), (1, 0), (0, 0), (2, 0), (1, 2), (0, 2),
        (2, 2)]
# mirrored block order (s2, s0, s1) for the second t-half: the rhs
# stream then stays in the same x copy-block across the u-boundary AND
# across the ctp-boundary, saving two block-switch penalties per c-pair
TAPS_M = [(1, 2), (0, 2), (2, 2), (1, 0), (0, 0), (2, 0), (1, 1),
          (0, 1), (2, 1)]

_cache = {}


def _build_nc():
    import concourse.bacc as bacc
    import concourse.mybir as mybir
    from concourse import tile

    f32 = mybir.dt.float32
    bf16 = mybir.dt.bfloat16
    fp8 = mybir.dt.float8e4
    DR = mybir.MatmulPerfMode.DoubleRow
    ALU = mybir.AluOpType

    nc = bacc.Bacc(trn_type="TRN2")

    w8_d = nc.dram_tensor("w8", [DT, P, CTP * 9 * 2 * P], fp8,
                          kind="ExternalInput")
    xp8_d = nc.dram_tensor("xp8", [CTP, P, XF], fp8, kind="ExternalInput")
    cb_d = nc.dram_tensor("cb", [P, DT], f32, kind="ExternalInput")
    sw_d = nc.dram_tensor("sw", [P, DT], bf16, kind="ExternalInput")
    out_d = nc.dram_tensor("out", [1, NPOS], f32, kind="ExternalOutput")

    with tile.TileContext(nc) as tc:
        with (
            tc.tile_pool(name="const", bufs=1) as cp,
            tc.tile_pool(name="wps_", bufs=3) as wp,
            tc.tile_pool(name="w3p_", bufs=1) as w3p,
            tc.tile_pool(name="rp", bufs=4) as rp,
            tc.tile_pool(name="cps", bufs=2, space="PSUM") as cps,
            tc.tile_pool(name="sps", bufs=1, space="PSUM") as sps,
            tc.tile_pool(name="mps", bufs=1, space="PSUM") as mps,
        ):
            # ---------- small constants (scalar HWDGE ring) ----------
            cb_sb = cp.tile([P, DT], f32)
            nc.scalar.dma_start(cb_sb[:], cb_d[:])
            sw_sb = cp.tile([P, DT], bf16)
            nc.scalar.dma_start(sw_sb[:], sw_d[:])
            # PE warm-up: dummy fp8 DR matmuls with no DMA deps run
            # during the DMA lead-in (HAM clock-gate ramp). Small tiles
            # + gpsimd memsets so the warm-up starts as early as
            # possible after the start barrier.
            WRF = NF
            wrm_in = cp.tile([P, 2, WRF], fp8)
            nc.gpsimd.memset(wrm_in[:], 0.0)
            wrm_w = cp.tile([P, 2, P], fp8)
            nc.gpsimd.memset(wrm_w[:], 1.0)
            wrm_ps = mps.tile([P, WRF], f32, tag="wm")
            for i in range(11):
                nc.tensor.matmul(wrm_ps[:], wrm_w[:], wrm_in[:],
                                 start=True, stop=True, perf_mode=DR)
            zero_sb = cp.tile([P, NF], f32)
            nc.gpsimd.memset(zero_sb[:], 0.0)

            xp8tiles = [cp.tile([P, XF], fp8, name=f"xp8_{i}")
                        for i in range(CTP)]

            def xview(ctp, dw):
                wd = 7 if dw == 1 else 6
                v = xp8tiles[ctp][:].rearrange("p (two q) -> p two q",
                                               two=2)
                vb = v[:, :, SOFF[dw]:SOFF[dw] + T * 7 * wd]
                return vb.rearrange("p two (t f) -> p two t f", t=T)

            s_ps = [sps.tile([1, NF], f32, tag=f"s{u}", name=f"s_ps{u}")
                    for u in range(NH)]

            def emit_score(dt, relu_tiles, us=(0, 1)):
                for u in us:
                    nc.tensor.matmul(s_ps[u][:], sw_sb[:, dt:dt + 1],
                                     relu_tiles[u][:],
                                     start=(dt == 0), stop=(dt == DT - 1))

            def emit_relu(dt, ps, u):
                # relu' = max(psum + 64*b, 0) -> bf16 on the DVE
                # (64x scale folded into sw on host)
                relu_t = rp.tile([P, NF], bf16, name=f"relu_{dt}_{u}",
                                 tag=f"relu{u}")
                nc.vector.scalar_tensor_tensor(
                    relu_t[:], ps[u][:], cb_sb[:, dt:dt + 1],
                    zero_sb[:],
                    op0=ALU.add, op1=ALU.max)
                return relu_t

            def emit_taps(pviews, w_ct, ctp, u, taps_u, start_ctp,
                          stop_ctp):
                for ti, (dh, dw) in enumerate(taps_u):
                    tap = dh * 3 + dw
                    wsl = w_ct[:, tap * 2 * P:(tap + 1) * 2 * P]
                    lhsT3 = wsl.rearrange("p (two q) -> p two q", two=2)
                    # full zero-trim: only valid out rows/cols
                    wd = 7 if dw == 1 else 6
                    oh0 = max(0, 1 - dh)
                    oh1 = min(H, H + 1 - dh)
                    ow0 = max(0, 1 - dw)
                    ow1 = min(W, W + 1 - dw)
                    ir0, ir1 = oh0 + dh - 1, oh1 + dh - 1
                    xv = xview(ctp, dw)
                    rhs = xv[:, :, u * TH:(u + 1) * TH, ir0 * wd:ir1 * wd]
                    nc.tensor.matmul(
                        pviews[u][:, :, oh0:oh1, ow0:ow1],
                        lhsT3, rhs,
                        start=(start_ctp and ti == 0),
                        stop=(stop_ctp and ti == len(TAPS) - 1),
                        perf_mode=DR, skip_group_check=True)

            def emit_conv_dt(dt):
                ps = [cps.tile([P, NF], f32, tag=f"cv{u}",
                               name=f"ps{dt}_{u}")
                      for u in range(NH)]
                pviews = [p[:].rearrange("p (t h w) -> p t h w",
                                         t=TH, h=H, w=W) for p in ps]
                for ctp in range(CTP):
                    # per-ctp weight chunk; paired with the ctp's x tile
                    # on dt0 so supply matches consumption.
                    w_ct = wp.tile([P, 9 * 2 * P], fp8, name="w_ct",
                                   tag=f"w_ct{ctp % 3}")
                    nc.sync.dma_start(
                        w_ct[:], w8_d[dt][:, ctp * 9 * 2 * P:
                                          (ctp + 1) * 9 * 2 * P])
                    if dt == 0:
                        nc.sync.dma_start(xp8tiles[ctp][:], xp8_d[ctp])
                    # u-outer: all 9 taps for t-half 0, then for t-half 1
                    # (psum bank switches once per 9 MMs, not every MM).
                    for u in range(NH):
                        taps_u = TAPS if (u == 0 or ctp == 0) else TAPS_M
                        emit_taps(pviews, w_ct, ctp, u, taps_u,
                                  start_ctp=(ctp == 0),
                                  stop_ctp=(ctp == CTP - 1))
                return ps

            def emit_conv_dt_last(dt, prev):
                # u-outer over the whole d-tile: conv(u0) for all ctps
                # (resident weight chunks), score(prev), conv(u1), so
                # relu(u0)+score(u0) overlap the u1 convolution.
                ps = [cps.tile([P, NF], f32, tag=f"cv{u}",
                               name=f"ps{dt}_{u}")
                      for u in range(NH)]
                pviews = [p[:].rearrange("p (t h w) -> p t h w",
                                         t=TH, h=H, w=W) for p in ps]
                w_cts = []
                for ctp in range(CTP):
                    w_ct = w3p.tile([P, 9 * 2 * P], fp8,
                                    name=f"w3_{ctp}", tag=f"w3_{ctp}")
                    nc.sync.dma_start(
                        w_ct[:], w8_d[dt][:, ctp * 9 * 2 * P:
                                          (ctp + 1) * 9 * 2 * P])
                    w_cts.append(w_ct)
                for ctp in range(CTP):
                    emit_taps(pviews, w_cts[ctp], ctp, 0,
                              TAPS, start_ctp=(ctp == 0),
                              stop_ctp=(ctp == CTP - 1))
                # relu(u0) on the DVE overlaps the u1 convolution; the
                # u0 score matmul then issues with its input ready.
                r0 = emit_relu(dt, ps, 0)
                emit_score(*prev)                      # score(dt2)
                for ctp in range(CTP):
                    taps_u = TAPS if ctp == 0 else TAPS_M
                    emit_taps(pviews, w_cts[ctp], ctp, 1,
                              taps_u, start_ctp=(ctp == 0),
                              stop_ctp=(ctp == CTP - 1))
                emit_score(dt, [r0], us=(0,))
                r1 = emit_relu(dt, ps, 1)
                emit_score(dt, [None, r1], us=(1,))

            # software-pipeline the PE queue: score MMs for dt are
            # enqueued after conv(dt+1), so the PE never waits on relu.
            prev = None
            for dt in range(DT - 1):
                ps = emit_conv_dt(dt)
                if prev is not None:
                    emit_score(*prev)
                relu_tiles = [emit_relu(dt, ps, u) for u in range(NH)]
                prev = (dt, relu_tiles)
            emit_conv_dt_last(DT - 1, prev)

            # ---------- epilogue: raw logits out ----------
            outs = cp.tile([1, NPOS], f32)
            nc.vector.tensor_copy(outs[0:1, 0:NF], s_ps[0][:])
            nc.vector.tensor_copy(outs[0:1, NF:NPOS], s_ps[1][:])
            nc.sync.dma_start(out_d[:], outs[:])

    nc.compile()
    return nc


def _prep_in_maps(x, x_fpv_pred, proj_weight, conv1_w, conv1_b, score_w,
                  score_b):
    import concourse.mybir as mybir
    bf16 = ml_dtypes.bfloat16
    fp8 = mybir.dt.np(mybir.dt.float8e4)

    # unpadded w-shifted copies per c-plane: s1 = raw x [T,7,7],
    # s0 = cols 0..5 [T,7,6], s2 = cols 1..6 [T,7,6]
    xr = np.asarray(x, np.float32).reshape(B, CTP, 2, P, T, H, W)
    xr = xr.transpose(0, 1, 3, 2, 4, 5, 6)      # [B,CTP,P,two,T,7,7]
    lead = (B, CTP, P, 2)
    b1 = xr.reshape(*lead, SPT1)
    b0 = np.ascontiguousarray(xr[..., 0:6]).reshape(*lead, SPT0)
    b2 = np.ascontiguousarray(xr[..., 1:7]).reshape(*lead, SPT0)
    xp8 = np.ascontiguousarray(
        np.concatenate([b1, b0, b2], axis=-1).reshape(B, CTP, P, XF)
    ).astype(fp8)

    w9 = np.asarray(conv1_w, np.float32).reshape(D, C, 9)
    # w8[dt, p, ((ctp*9 + tap)*2 + two)*P + q]
    #   = WSCALE * conv1_w[dt*P+q, (2*ctp+two)*P+p, tap]
    w8 = np.ascontiguousarray(
        (w9 * WSCALE).reshape(DT, P, CTP, 2, P, 9).transpose(0, 4, 2, 5, 3, 1)
        .reshape(DT, P, CTP * 9 * 2 * P)).astype(fp8)

    # bias pre-scaled by WSCALE (the relu keeps the x64 scale; the
    # score weights absorb the 1/64)
    cb = np.ascontiguousarray(
        (np.asarray(conv1_b, np.float32) * WSCALE).reshape(DT, P).T)
    sw = np.ascontiguousarray(
        (np.asarray(score_w, np.float32) / WSCALE).reshape(DT, P).T
    ).astype(bf16)

    in_maps = []
    for b in range(B):
        in_maps.append({
            "xp8": xp8[b],
            "w8": w8,
            "cb": cb,
            "sw": sw,
        })
    return in_maps


def _host_cam_gt(x, x_fpv_pred, proj_weight):
    """Exact numpy replica of the reference CAM target (stop-gradient)."""
    xf = np.asarray(x, np.float32).reshape(B, C, NPOS)
    top_cls = np.argmax(np.asarray(x_fpv_pred, np.float32), axis=1)
    rows = np.asarray(proj_weight, np.float32)[top_cls]        # [B, C]
    cam = np.einsum('bc,bcn->bn', rows, xf)                    # [B, NPOS]
    cmin = cam.min(axis=1, keepdims=True)
    cmax = cam.max(axis=1, keepdims=True)
    cam_n = (cam - cmin) / (cmax - cmin)
    y = np.zeros_like(cam_n)
    # top-392 of 784 by value (ties measure-zero for random floats)
    idx = np.argpartition(-cam_n, N_TOKEN - 1, axis=1)[:, :N_TOKEN]
    np.put_along_axis(y, idx, np.take_along_axis(cam_n, idx, axis=1),
                      axis=1)
    return y


def run(inputs, trace=False):
    """Build (cached), run on 8 cores, return (loss, BassKernelResults)."""
    from concourse.bass_utils import run_bass_kernel_spmd

    if "nc" not in _cache:
        _cache["nc"] = _build_nc()
    nc = _cache["nc"]
    in_maps = _prep_in_maps(**inputs)
    res = run_bass_kernel_spmd(nc, in_maps, core_ids=list(range(B)),
                               trace=trace)
    y = _host_cam_gt(inputs["x"], inputs["x_fpv_pred"],
                     inputs["proj_weight"])
    sb = float(np.asarray(inputs["score_b"], np.float32).reshape(-1)[0])
    logits = np.stack([
        np.asarray(res.results[b]["out"], np.float32).reshape(NPOS)
        for b in range(B)]) + sb                               # [B, 784]
    xv = logits.astype(np.float64)
    yv = y.astype(np.float64)
    bce = (1.0 - yv) * xv + np.logaddexp(0.0, -xv)
    loss = np.float32(bce.mean())
    return loss, res


def kernel(**inputs):
    loss, _ = run(inputs, trace=False)
    return loss


# revision 15
# speedup vs baseline: 1.0172x; 1.0172x over previous
"""Trainium2 Bass kernel for nn_CAMLocalHead (CAM target + conv head + BCE).

v3: conv head on device at fp8-DoubleRow rate; cheap parts (argmax, CAM
row matvec, min-max norm, top-k scatter, BCE reduce) on host.

Sharding: one sample per core (8 cores). Per core:
  - Conv3d(2048->512, 1x3x3, pad 011) as 9 shifted fp8 DoubleRow matmuls
    per (d-tile, c-pair) accumulating in PSUM. x stored as 3 w-shifted
    UNPADDED copies of widths (7,6,6): edge taps are trimmed to their
    valid output region (strided psum out APs) so no zero row or column
    is ever streamed.
  - Weights pre-scaled x64 into e4m3 range. Bias+ReLU fused into ONE
    DVE scalar_tensor_tensor per (d-tile, t-half): max(psum + 64*b, 0)
    -> bf16 (the 1/64 un-scale is folded into the score weights on
    host). Scalar/ACT engine is not used at all.
  - Score conv = one bf16 matmul per (d-tile, t-half) into [1, 392]
    psums accumulated across d-tiles. Score matmuls are emitted one
    d-tile late so the PE never stalls waiting on the relu.
  - Last d-tile runs u-outer (all c-pairs for t-half 0, then t-half 1)
    from a resident 8-chunk weight pool, so relu+score of half 0
    overlap the half-1 convolution and the tail shrinks.
  - Output: raw logits [1, 784] (score_b added on host). Epilogue:
    psum -> SBUF copies on DVE + GpSimd in parallel, one DMA out.

Host: cam = proj_weight[argmax(pred)] @ x (exact f32), top-392 mask and
scatter, then BCE mean over all samples in f64.
"""
import sys

for _p in ("/opt/trn_rl_repo", "/opt/pypackages"):
    if _p not in sys.path:
        sys.path.append(_p)

import numpy as np
import ml_dtypes

# Problem dims (hardcoded per spec)
B, C, T, H, W = 8, 2048, 16, 7, 7
K, D = 400, 512
N_TOKEN = 392
P = 128
CT = C // P          # 16 c-tiles
CTP = CT // 2        # 8 c-tile pairs (DoubleRow)
DT = D // P          # 4 d-tiles
NH = 2               # t halves (t 0..7, 8..15)
TH = T // NH         # 8
NF = TH * H * W      # 392 positions per half
NPOS = T * H * W     # 784
CW = (7, 6, 6)       # copy widths for dw = 1, 0, 2 (stored s1, s0, s2)
SPT1 = T * 7 * CW[0]  # 784:  s=1 copy (raw x), all t
SPT0 = T * 7 * CW[1]  # 672:  s=0 copy, all t
SOFF = {1: 0, 0: SPT1, 2: SPT1 + SPT0}   # offsets within one c-plane
CPL = SPT1 + 2 * SPT0  # 2128: one c-plane's three copies
XF = 2 * CPL         # 4256: free size of one fp8 x pair-tile
WSCALE = 64.0

# taps ordered so ctp0 starts with the full-coverage center tap (its
# start=True matmul initializes the whole psum region) and the s=1 copy
# is needed first, then s=0, then s=2 (matching the x DMA split order).
TAPS = [(1, 1), (0, 1), (2, 1), (1, 0), (0, 0), (2, 0), (1, 2), (0, 2),
        (2, 2)]
# mirrored block order (s2, s0, s1) for the second t-half: the rhs
# stream then stays in the same x copy-block across the u-boundary AND
# across the ctp-boundary, saving two block-switch penalties per c-pair
TAPS_M = [(1, 2), (0, 2), (2, 2), (1, 0), (0, 0), (2, 0), (1, 1),
          (0, 1), (2, 1)]

_cache = {}


def _build_nc():
    import concourse.bacc as bacc
    import concourse.mybir as mybir
    from concourse import tile

    f32 = mybir.dt.float32
    bf16 = mybir.dt.bfloat16
    fp8 = mybir.dt.float8e4
    DR = mybir.MatmulPerfMode.DoubleRow
    ALU = mybir.AluOpType

    nc = bacc.Bacc(trn_type="TRN2")

    w8_d = nc.dram_tensor("w8", [DT, P, CTP * 9 * 2 * P], fp8,
                          kind="ExternalInput")
    xp8_d = nc.dram_tensor("xp8", [CTP, P, XF], fp8, kind="ExternalInput")
    cb_d = nc.dram_tensor("cb", [P, DT], f32, kind="ExternalInput")
    sw_d = nc.dram_tensor("sw", [P, DT], bf16, kind="ExternalInput")
    out_d = nc.dram_tensor("out", [1, NPOS], f32, kind="ExternalOutput")

    with tile.TileContext(nc) as tc:
        with (
            tc.tile_pool(name="const", bufs=1) as cp,
            tc.tile_pool(name="wps_", bufs=3) as wp,
            tc.tile_pool(name="w3p_", bufs=1) as w3p,
            tc.tile_pool(name="rp", bufs=4) as rp,
            tc.tile_pool(name="cps", bufs=2, space="PSUM") as cps,
            tc.tile_pool(name="sps", bufs=1, space="PSUM") as sps,
            tc.tile_pool(name="mps", bufs=1, space="PSUM") as mps,
        ):
            # ---------- small constants (scalar HWDGE ring) ----------
            cb_sb = cp.tile([P, DT], f32)
            nc.scalar.dma_start(cb_sb[:], cb_d[:])
            sw_sb = cp.tile([P, DT], bf16)
            nc.scalar.dma_start(sw_sb[:], sw_d[:])
            # PE warm-up: dummy fp8 DR matmuls with no DMA deps run
            # during the DMA lead-in (HAM clock-gate ramp). Small tiles
            # + gpsimd memsets so the warm-up starts as early as
            # possible after the start barrier.
            WRF = NF
            wrm_in = cp.tile([P, 2, WRF], fp8)
            nc.gpsimd.memset(wrm_in[:], 0.0)
            wrm_w = cp.tile([P, 2, P], fp8)
            nc.gpsimd.memset(wrm_w[:], 1.0)
            wrm_ps = mps.tile([P, WRF], f32, tag="wm")
            for i in range(10):
                nc.tensor.matmul(wrm_ps[:], wrm_w[:], wrm_in[:],
                                 start=True, stop=True, perf_mode=DR)
            zero_sb = cp.tile([P, NF], f32)
            nc.gpsimd.memset(zero_sb[:], 0.0)

            xp8tiles = [cp.tile([P, XF], fp8, name=f"xp8_{i}")
                        for i in range(CTP)]

            def xview(ctp, dw):
                # block-major x layout: [s1(two,784) | s0(two,672) |
                # s2(two,672)] per partition
                wd = 7 if dw == 1 else 6
                vb = xp8tiles[ctp][:, 2 * SOFF[dw]:
                                   2 * (SOFF[dw] + T * 7 * wd)]
                v = vb.rearrange("p (two q) -> p two q", two=2)
                return v.rearrange("p two (t f) -> p two t f", t=T)

            s_ps = [sps.tile([1, NF], f32, tag=f"s{u}", name=f"s_ps{u}")
                    for u in range(NH)]

            def emit_score(dt, relu_tiles, us=(0, 1)):
                for u in us:
                    nc.tensor.matmul(s_ps[u][:], sw_sb[:, dt:dt + 1],
                                     relu_tiles[u][:],
                                     start=(dt == 0), stop=(dt == DT - 1))

            def emit_relu(dt, ps, u):
                # relu' = max(psum + 64*b, 0) -> bf16 on the DVE
                # (64x scale folded into sw on host)
                relu_t = rp.tile([P, NF], bf16, name=f"relu_{dt}_{u}",
                                 tag=f"relu{u}")
                nc.vector.scalar_tensor_tensor(
                    relu_t[:], ps[u][:], cb_sb[:, dt:dt + 1],
                    zero_sb[:],
                    op0=ALU.add, op1=ALU.max)
                return relu_t

            def emit_taps(pviews, w_ct, ctp, u, taps_u, start_ctp,
                          stop_ctp):
                for ti, (dh, dw) in enumerate(taps_u):
                    tap = dh * 3 + dw
                    wsl = w_ct[:, tap * 2 * P:(tap + 1) * 2 * P]
                    lhsT3 = wsl.rearrange("p (two q) -> p two q", two=2)
                    # full zero-trim: only valid out rows/cols
                    wd = 7 if dw == 1 else 6
                    oh0 = max(0, 1 - dh)
                    oh1 = min(H, H + 1 - dh)
                    ow0 = max(0, 1 - dw)
                    ow1 = min(W, W + 1 - dw)
                    ir0, ir1 = oh0 + dh - 1, oh1 + dh - 1
                    xv = xview(ctp, dw)
                    rhs = xv[:, :, u * TH:(u + 1) * TH, ir0 * wd:ir1 * wd]
                    nc.tensor.matmul(
                        pviews[u][:, :, oh0:oh1, ow0:ow1],
                        lhsT3, rhs,
                        start=(start_ctp and ti == 0),
                        stop=(stop_ctp and ti == len(TAPS) - 1),
                        perf_mode=DR, skip_group_check=True)

            def emit_conv_dt(dt):
                ps = [cps.tile([P, NF], f32, tag=f"cv{u}",
                               name=f"ps{dt}_{u}")
                      for u in range(NH)]
                pviews = [p[:].rearrange("p (t h w) -> p t h w",
                                         t=TH, h=H, w=W) for p in ps]
                for ctp in range(CTP):
                    # per-ctp weight chunk; paired with the ctp's x tile
                    # on dt0 so supply matches consumption.
                    w_ct = wp.tile([P, 9 * 2 * P], fp8, name="w_ct",
                                   tag=f"w_ct{ctp % 3}")
                    nc.sync.dma_start(
                        w_ct[:], w8_d[dt][:, ctp * 9 * 2 * P:
                                          (ctp + 1) * 9 * 2 * P])
                    if dt == 0:
                        # split: s1 block first (first 3 taps), rest after
                        nc.sync.dma_start(
                            xp8tiles[ctp][:, 0:2 * SPT1],
                            xp8_d[ctp][:, 0:2 * SPT1])
                        nc.sync.dma_start(
                            xp8tiles[ctp][:, 2 * SPT1:],
                            xp8_d[ctp][:, 2 * SPT1:])
                    # u-outer: all 9 taps for t-half 0, then for t-half 1
                    # (psum bank switches once per 9 MMs, not every MM).
                    for u in range(NH):
                        taps_u = TAPS if (u == 0 or ctp == 0) else TAPS_M
                        emit_taps(pviews, w_ct, ctp, u, taps_u,
                                  start_ctp=(ctp == 0),
                                  stop_ctp=(ctp == CTP - 1))
                return ps

            def emit_conv_dt_last(dt, prev):
                # u-outer over the whole d-tile: conv(u0) for all ctps
                # (resident weight chunks), score(prev), conv(u1), so
                # relu(u0)+score(u0) overlap the u1 convolution.
                ps = [cps.tile([P, NF], f32, tag=f"cv{u}",
                               name=f"ps{dt}_{u}")
                      for u in range(NH)]
                pviews = [p[:].rearrange("p (t h w) -> p t h w",
                                         t=TH, h=H, w=W) for p in ps]
                w_cts = []
                for ctp in range(CTP):
                    w_ct = w3p.tile([P, 9 * 2 * P], fp8,
                                    name=f"w3_{ctp}", tag=f"w3_{ctp}")
                    nc.sync.dma_start(
                        w_ct[:], w8_d[dt][:, ctp * 9 * 2 * P:
                                          (ctp + 1) * 9 * 2 * P])
                    w_cts.append(w_ct)
                for ctp in range(CTP):
                    emit_taps(pviews, w_cts[ctp], ctp, 0,
                              TAPS, start_ctp=(ctp == 0),
                              stop_ctp=(ctp == CTP - 1))
                # relu(u0) on the DVE overlaps the u1 convolution; the
                # u0 score matmul then issues with its input ready.
                r0 = emit_relu(dt, ps, 0)
                emit_score(*prev)                      # score(dt2)
                for ctp in range(CTP):
                    taps_u = TAPS if ctp == 0 else TAPS_M
                    emit_taps(pviews, w_cts[ctp], ctp, 1,
                              taps_u, start_ctp=(ctp == 0),
                              stop_ctp=(ctp == CTP - 1))
                emit_score(dt, [r0], us=(0,))
                r1 = emit_relu(dt, ps, 1)
                emit_score(dt, [None, r1], us=(1,))

            # software-pipeline the PE queue: score MMs for dt are
            # enqueued after conv(dt+1), so the PE never waits on relu.
            prev = None
            for dt in range(DT - 1):
                ps = emit_conv_dt(dt)
                if prev is not None:
                    emit_score(*prev)
                relu_tiles = [emit_relu(dt, ps, u) for u in range(NH)]
                prev = (dt, relu_tiles)
            emit_conv_dt_last(DT - 1, prev)

            # ---------- epilogue: raw logits out ----------
            outs = cp.tile([1, NPOS], f32)
            nc.vector.tensor_copy(outs[0:1, 0:NF], s_ps[0][:])
            nc.vector.tensor_copy(outs[0:1, NF:NPOS], s_ps[1][:])
            nc.sync.dma_start(out_d[:], outs[:])

    nc.compile()
    return nc


def _prep_in_maps(x, x_fpv_pred, proj_weight, conv1_w, conv1_b, score_w,
                  score_b):
    import concourse.mybir as mybir
    bf16 = ml_dtypes.bfloat16
    fp8 = mybir.dt.np(mybir.dt.float8e4)

    # unpadded w-shifted copies per c-plane: s1 = raw x [T,7,7],
    # s0 = cols 0..5 [T,7,6], s2 = cols 1..6 [T,7,6]
    xr = np.asarray(x, np.float32).reshape(B, CTP, 2, P, T, H, W)
    xr = xr.transpose(0, 1, 3, 2, 4, 5, 6)      # [B,CTP,P,two,T,7,7]
    # block-major: per partition [s1(two,784) | s0(two,672) | s2(two,672)]
    lead = (B, CTP, P)
    b1 = xr.reshape(*lead, 2 * SPT1)
    b0 = np.ascontiguousarray(xr[..., 0:6]).reshape(*lead, 2 * SPT0)
    b2 = np.ascontiguousarray(xr[..., 1:7]).reshape(*lead, 2 * SPT0)
    xp8 = np.ascontiguousarray(
        np.concatenate([b1, b0, b2], axis=-1).reshape(B, CTP, P, XF)
    ).astype(fp8)

    w9 = np.asarray(conv1_w, np.float32).reshape(D, C, 9)
    # w8[dt, p, ((ctp*9 + tap)*2 + two)*P + q]
    #   = WSCALE * conv1_w[dt*P+q, (2*ctp+two)*P+p, tap]
    w8 = np.ascontiguousarray(
        (w9 * WSCALE).reshape(DT, P, CTP, 2, P, 9).transpose(0, 4, 2, 5, 3, 1)
        .reshape(DT, P, CTP * 9 * 2 * P)).astype(fp8)

    # bias pre-scaled by WSCALE (the relu keeps the x64 scale; the
    # score weights absorb the 1/64)
    cb = np.ascontiguousarray(
        (np.asarray(conv1_b, np.float32) * WSCALE).reshape(DT, P).T)
    sw = np.ascontiguousarray(
        (np.asarray(score_w, np.float32) / WSCALE).reshape(DT, P).T
    ).astype(bf16)

    in_maps = []
    for b in range(B):
        in_maps.append({
            "xp8": xp8[b],
            "w8": w8,
            "cb": cb,
            "sw": sw,
        })
    return in_maps


def _host_cam_gt(x, x_fpv_pred, proj_weight):
    """Exact numpy replica of the reference CAM target (stop-gradient)."""
    xf = np.asarray(x, np.float32).reshape(B, C, NPOS)
    top_cls = np.argmax(np.asarray(x_fpv_pred, np.float32), axis=1)
    rows = np.asarray(proj_weight, np.float32)[top_cls]        # [B, C]
    cam = np.einsum('bc,bcn->bn', rows, xf)                    # [B, NPOS]
    cmin = cam.min(axis=1, keepdims=True)
    cmax = cam.max(axis=1, keepdims=True)
    cam_n = (cam - cmin) / (cmax - cmin)
    y = np.zeros_like(cam_n)
    # top-392 of 784 by value (ties measure-zero for random floats)
    idx = np.argpartition(-cam_n, N_TOKEN - 1, axis=1)[:, :N_TOKEN]
    np.put_along_axis(y, idx, np.take_along_axis(cam_n, idx, axis=1),
                      axis=1)
    return y


def run(inputs, trace=False):
    """Build (cached), run on 8 cores, return (loss, BassKernelResults)."""
    from concourse.bass_utils import run_bass_kernel_spmd

    if "nc" not in _cache:
        _cache["nc"] = _build_nc()
    nc = _cache["nc"]
    in_maps = _prep_in_maps(**inputs)
    res = run_bass_kernel_spmd(nc, in_maps, core_ids=list(range(B)),
                               trace=trace)
    y = _host_cam_gt(inputs["x"], inputs["x_fpv_pred"],
                     inputs["proj_weight"])
    sb = float(np.asarray(inputs["score_b"], np.float32).reshape(-1)[0])
    logits = np.stack([
        np.asarray(res.results[b]["out"], np.float32).reshape(NPOS)
        for b in range(B)]) + sb                               # [B, 784]
    xv = logits.astype(np.float64)
    yv = y.astype(np.float64)
    bce = (1.0 - yv) * xv + np.logaddexp(0.0, -xv)
    loss = np.float32(bce.mean())
    return loss, res


def kernel(**inputs):
    loss, _ = run(inputs, trace=False)
    return loss


# revision 18
# speedup vs baseline: 1.0215x; 1.0043x over previous
"""Trainium2 Bass kernel for nn_CAMLocalHead (CAM target + conv head + BCE).

v3: conv head on device at fp8-DoubleRow rate; cheap parts (argmax, CAM
row matvec, min-max norm, top-k scatter, BCE reduce) on host.

Sharding: one sample per core (8 cores). Per core:
  - Conv3d(2048->512, 1x3x3, pad 011) as 9 shifted fp8 DoubleRow matmuls
    per (d-tile, c-pair) accumulating in PSUM. x stored as 3 w-shifted
    UNPADDED copies of widths (7,6,6): edge taps are trimmed to their
    valid output region (strided psum out APs) so no zero row or column
    is ever streamed.
  - Weights pre-scaled x64 into e4m3 range. Bias+ReLU fused into ONE
    DVE scalar_tensor_tensor per (d-tile, t-half): max(psum + 64*b, 0)
    -> bf16 (the 1/64 un-scale is folded into the score weights on
    host). Scalar/ACT engine is not used at all.
  - Score conv = one bf16 matmul per (d-tile, t-half) into [1, 392]
    psums accumulated across d-tiles. Score matmuls are emitted one
    d-tile late so the PE never stalls waiting on the relu.
  - Last d-tile runs u-outer (all c-pairs for t-half 0, then t-half 1)
    from a resident 8-chunk weight pool, so relu+score of half 0
    overlap the half-1 convolution and the tail shrinks.
  - Output: raw logits [1, 784] (score_b added on host). Epilogue:
    psum -> SBUF copies on DVE + GpSimd in parallel, one DMA out.

Host: cam = proj_weight[argmax(pred)] @ x (exact f32), top-392 mask and
scatter, then BCE mean over all samples in f64.
"""
import sys

for _p in ("/opt/trn_rl_repo", "/opt/pypackages"):
    if _p not in sys.path:
        sys.path.append(_p)

import numpy as np
import ml_dtypes

# Problem dims (hardcoded per spec)
B, C, T, H, W = 8, 2048, 16, 7, 7
K, D = 400, 512
N_TOKEN = 392
P = 128
CT = C // P          # 16 c-tiles
CTP = CT // 2        # 8 c-tile pairs (DoubleRow)
DT = D // P          # 4 d-tiles
NH = 2               # t halves (t 0..7, 8..15)
TH = T // NH         # 8
NF = TH * H * W      # 392 positions per half
NPOS = T * H * W     # 784
CW = (7, 6, 6)       # copy widths for dw = 1, 0, 2 (stored s1, s0, s2)
SPT1 = T * 7 * CW[0]  # 784:  s=1 copy (raw x), all t
SPT0 = T * 7 * CW[1]  # 672:  s=0 copy, all t
SOFF = {1: 0, 0: SPT1, 2: SPT1 + SPT0}   # offsets within one c-plane
CPL = SPT1 + 2 * SPT0  # 2128: one c-plane's three copies
XF = 2 * CPL         # 4256: free size of one fp8 x pair-tile
WSCALE = 64.0

# taps ordered so ctp0 starts with the full-coverage center tap (its
# start=True matmul initializes the whole psum region) and the s=1 copy
# is needed first, then s=0, then s=2 (matching the x DMA split order).
TAPS = [(1, 1), (0, 1), (2, 1), (1, 0), (0, 0), (2, 0), (1, 2), (0, 2),
        (2, 2)]
# mirrored block order (s2, s0, s1) for the second t-half: the rhs
# stream then stays in the same x copy-block across the u-boundary AND
# across the ctp-boundary, saving two block-switch penalties per c-pair
TAPS_M = [(1, 2), (0, 2), (2, 2), (1, 0), (0, 0), (2, 0), (1, 1),
          (0, 1), (2, 1)]
# corner-first variants for non-start groups: the first tap of a new
# copy-block pays a ~20-25ns AP-restart penalty; leading with a 288-col
# corner tap hides it under the LDWEIGHTS floor that tap pays anyway.
TAPS2 = [(1, 1), (0, 1), (2, 1), (0, 0), (2, 0), (1, 0), (0, 2),
         (2, 2), (1, 2)]
TAPS2_M = [(0, 2), (2, 2), (1, 2), (0, 0), (2, 0), (1, 0), (1, 1),
           (0, 1), (2, 1)]

_cache = {}


def _build_nc():
    import concourse.bacc as bacc
    import concourse.mybir as mybir
    from concourse import tile

    f32 = mybir.dt.float32
    bf16 = mybir.dt.bfloat16
    fp8 = mybir.dt.float8e4
    DR = mybir.MatmulPerfMode.DoubleRow
    ALU = mybir.AluOpType

    nc = bacc.Bacc(trn_type="TRN2")

    w8_d = nc.dram_tensor("w8", [DT, P, CTP * 9 * 2 * P], fp8,
                          kind="ExternalInput")
    xp8_d = nc.dram_tensor("xp8", [CTP, P, XF], fp8, kind="ExternalInput")
    cb_d = nc.dram_tensor("cb", [P, DT], f32, kind="ExternalInput")
    sw_d = nc.dram_tensor("sw", [P, DT], bf16, kind="ExternalInput")
    out_d = nc.dram_tensor("out", [1, NPOS], f32, kind="ExternalOutput")

    with tile.TileContext(nc) as tc:
        with (
            tc.tile_pool(name="const", bufs=1) as cp,
            tc.tile_pool(name="wps_", bufs=3) as wp,
            tc.tile_pool(name="w3p_", bufs=1) as w3p,
            tc.tile_pool(name="rp", bufs=4) as rp,
            tc.tile_pool(name="cps", bufs=2, space="PSUM") as cps,
            tc.tile_pool(name="sps", bufs=1, space="PSUM") as sps,
            tc.tile_pool(name="mps", bufs=1, space="PSUM") as mps,
        ):
            # ---------- small constants (scalar HWDGE ring) ----------
            cb_sb = cp.tile([P, DT], f32)
            nc.scalar.dma_start(cb_sb[:], cb_d[:])
            sw_sb = cp.tile([P, DT], bf16)
            nc.scalar.dma_start(sw_sb[:], sw_d[:])
            # PE warm-up: dummy fp8 DR matmuls with no DMA deps run
            # during the DMA lead-in (HAM clock-gate ramp). Small tiles
            # + gpsimd memsets so the warm-up starts as early as
            # possible after the start barrier.
            WRF = NF
            wrm_in = cp.tile([P, 2, WRF], fp8)
            nc.gpsimd.memset(wrm_in[:], 0.0)
            wrm_w = cp.tile([P, 2, P], fp8)
            nc.gpsimd.memset(wrm_w[:], 1.0)
            wrm_ps = mps.tile([P, WRF], f32, tag="wm")
            for i in range(10):
                nc.tensor.matmul(wrm_ps[:], wrm_w[:], wrm_in[:],
                                 start=True, stop=True, perf_mode=DR)
            zero_sb = cp.tile([P, NF], f32)
            nc.gpsimd.memset(zero_sb[:], 0.0)

            xp8tiles = [cp.tile([P, XF], fp8, name=f"xp8_{i}")
                        for i in range(CTP)]

            def xview(ctp, dw):
                # block-major x layout: [s1(two,784) | s0(two,672) |
                # s2(two,672)] per partition
                wd = 7 if dw == 1 else 6
                vb = xp8tiles[ctp][:, 2 * SOFF[dw]:
                                   2 * (SOFF[dw] + T * 7 * wd)]
                v = vb.rearrange("p (two q) -> p two q", two=2)
                return v.rearrange("p two (t f) -> p two t f", t=T)

            s_ps = [sps.tile([1, NF], f32, tag=f"s{u}", name=f"s_ps{u}")
                    for u in range(NH)]

            def emit_score(dt, relu_tiles, us=(0, 1)):
                for u in us:
                    nc.tensor.matmul(s_ps[u][:], sw_sb[:, dt:dt + 1],
                                     relu_tiles[u][:],
                                     start=(dt == 0), stop=(dt == DT - 1))

            def emit_relu(dt, ps, u):
                # relu' = max(psum + 64*b, 0) -> bf16 on the DVE
                # (64x scale folded into sw on host)
                relu_t = rp.tile([P, NF], bf16, name=f"relu_{dt}_{u}",
                                 tag=f"relu{u}")
                nc.vector.scalar_tensor_tensor(
                    relu_t[:], ps[u][:], cb_sb[:, dt:dt + 1],
                    zero_sb[:],
                    op0=ALU.add, op1=ALU.max)
                return relu_t

            def emit_taps(pviews, w_ct, ctp, u, taps_u, start_ctp,
                          stop_ctp):
                for ti, (dh, dw) in enumerate(taps_u):
                    tap = dh * 3 + dw
                    wsl = w_ct[:, tap * 2 * P:(tap + 1) * 2 * P]
                    lhsT3 = wsl.rearrange("p (two q) -> p two q", two=2)
                    # full zero-trim: only valid out rows/cols
                    wd = 7 if dw == 1 else 6
                    oh0 = max(0, 1 - dh)
                    oh1 = min(H, H + 1 - dh)
                    ow0 = max(0, 1 - dw)
                    ow1 = min(W, W + 1 - dw)
                    ir0, ir1 = oh0 + dh - 1, oh1 + dh - 1
                    xv = xview(ctp, dw)
                    rhs = xv[:, :, u * TH:(u + 1) * TH, ir0 * wd:ir1 * wd]
                    nc.tensor.matmul(
                        pviews[u][:, :, oh0:oh1, ow0:ow1],
                        lhsT3, rhs,
                        start=(start_ctp and ti == 0),
                        stop=(stop_ctp and ti == len(TAPS) - 1),
                        perf_mode=DR, skip_group_check=True)

            def emit_conv_dt(dt):
                ps = [cps.tile([P, NF], f32, tag=f"cv{u}",
                               name=f"ps{dt}_{u}")
                      for u in range(NH)]
                pviews = [p[:].rearrange("p (t h w) -> p t h w",
                                         t=TH, h=H, w=W) for p in ps]
                for ctp in range(CTP):
                    # per-ctp weight chunk; paired with the ctp's x tile
                    # on dt0 so supply matches consumption.
                    w_ct = wp.tile([P, 9 * 2 * P], fp8, name="w_ct",
                                   tag=f"w_ct{ctp % 3}")
                    nc.sync.dma_start(
                        w_ct[:], w8_d[dt][:, ctp * 9 * 2 * P:
                                          (ctp + 1) * 9 * 2 * P])
                    if dt == 0:
                        # split: s1 block first (first 3 taps), rest after
                        nc.sync.dma_start(
                            xp8tiles[ctp][:, 0:2 * SPT1],
                            xp8_d[ctp][:, 0:2 * SPT1])
                        nc.sync.dma_start(
                            xp8tiles[ctp][:, 2 * SPT1:],
                            xp8_d[ctp][:, 2 * SPT1:])
                    # u-outer: all 9 taps for t-half 0, then for t-half 1
                    # (psum bank switches once per 9 MMs, not every MM).
                    # ctp0 keeps center-first TAPS (its start=True matmul
                    # must cover the whole psum region).
                    for u in range(NH):
                        if ctp == 0:
                            taps_u = TAPS
                        else:
                            taps_u = TAPS2 if u == 0 else TAPS2_M
                        emit_taps(pviews, w_ct, ctp, u, taps_u,
                                  start_ctp=(ctp == 0),
                                  stop_ctp=(ctp == CTP - 1))
                return ps

            def emit_conv_dt_last(dt, prev):
                # u-outer over the whole d-tile: conv(u0) for all ctps
                # (resident weight chunks), score(prev), conv(u1), so
                # relu(u0)+score(u0) overlap the u1 convolution.
                ps = [cps.tile([P, NF], f32, tag=f"cv{u}",
                               name=f"ps{dt}_{u}")
                      for u in range(NH)]
                pviews = [p[:].rearrange("p (t h w) -> p t h w",
                                         t=TH, h=H, w=W) for p in ps]
                w_cts = []
                for ctp in range(CTP):
                    w_ct = w3p.tile([P, 9 * 2 * P], fp8,
                                    name=f"w3_{ctp}", tag=f"w3_{ctp}")
                    nc.sync.dma_start(
                        w_ct[:], w8_d[dt][:, ctp * 9 * 2 * P:
                                          (ctp + 1) * 9 * 2 * P])
                    w_cts.append(w_ct)
                for ctp in range(CTP):
                    emit_taps(pviews, w_cts[ctp], ctp, 0,
                              TAPS if ctp == 0 else TAPS2,
                              start_ctp=(ctp == 0),
                              stop_ctp=(ctp == CTP - 1))
                # relu(u0) on the DVE overlaps the u1 convolution; the
                # u0 score matmul then issues with its input ready.
                r0 = emit_relu(dt, ps, 0)
                emit_score(*prev)                      # score(dt2)
                for ctp in range(CTP):
                    taps_u = TAPS if ctp == 0 else TAPS2_M
                    emit_taps(pviews, w_cts[ctp], ctp, 1,
                              taps_u, start_ctp=(ctp == 0),
                              stop_ctp=(ctp == CTP - 1))
                emit_score(dt, [r0], us=(0,))
                r1 = emit_relu(dt, ps, 1)
                emit_score(dt, [None, r1], us=(1,))

            # software-pipeline the PE queue: score MMs for dt are
            # enqueued after conv(dt+1), so the PE never waits on relu.
            prev = None
            for dt in range(DT - 1):
                ps = emit_conv_dt(dt)
                if prev is not None:
                    emit_score(*prev)
                relu_tiles = [emit_relu(dt, ps, u) for u in range(NH)]
                prev = (dt, relu_tiles)
            emit_conv_dt_last(DT - 1, prev)

            # ---------- epilogue: raw logits out ----------
            outs = cp.tile([1, NPOS], f32)
            nc.vector.tensor_copy(outs[0:1, 0:NF], s_ps[0][:])
            nc.vector.tensor_copy(outs[0:1, NF:NPOS], s_ps[1][:])
            nc.sync.dma_start(out_d[:], outs[:])

    nc.compile()
    return nc


def _prep_in_maps(x, x_fpv_pred, proj_weight, conv1_w, conv1_b, score_w,
                  score_b):
    import concourse.mybir as mybir
    bf16 = ml_dtypes.bfloat16
    fp8 = mybir.dt.np(mybir.dt.float8e4)

    # unpadded w-shifted copies per c-plane: s1 = raw x [T,7,7],
    # s0 = cols 0..5 [T,7,6], s2 = cols 1..6 [T,7,6]
    xr = np.asarray(x, np.float32).reshape(B, CTP, 2, P, T, H, W)
    xr = xr.transpose(0, 1, 3, 2, 4, 5, 6)      # [B,CTP,P,two,T,7,7]
    # block-major: per partition [s1(two,784) | s0(two,672) | s2(two,672)]
    lead = (B, CTP, P)
    b1 = xr.reshape(*lead, 2 * SPT1)
    b0 = np.ascontiguousarray(xr[..., 0:6]).reshape(*lead, 2 * SPT0)
    b2 = np.ascontiguousarray(xr[..., 1:7]).reshape(*lead, 2 * SPT0)
    xp8 = np.ascontiguousarray(
        np.concatenate([b1, b0, b2], axis=-1).reshape(B, CTP, P, XF)
    ).astype(fp8)

    w9 = np.asarray(conv1_w, np.float32).reshape(D, C, 9)
    # w8[dt, p, ((ctp*9 + tap)*2 + two)*P + q]
    #   = WSCALE * conv1_w[dt*P+q, (2*ctp+two)*P+p, tap]
    w8 = np.ascontiguousarray(
        (w9 * WSCALE).reshape(DT, P, CTP, 2, P, 9).transpose(0, 4, 2, 5, 3, 1)
        .reshape(DT, P, CTP * 9 * 2 * P)).astype(fp8)

    # bias pre-scaled by WSCALE (the relu keeps the x64 scale; the
    # score weights absorb the 1/64)
    cb = np.ascontiguousarray(
        (np.asarray(conv1_b, np.float32) * WSCALE).reshape(DT, P).T)
    sw = np.ascontiguousarray(
        (np.asarray(score_w, np.float32) / WSCALE).reshape(DT, P).T
    ).astype(bf16)

    in_maps = []
    for b in range(B):
        in_maps.append({
            "xp8": xp8[b],
            "w8": w8,
            "cb": cb,
            "sw": sw,
        })
    return in_maps


def _host_cam_gt(x, x_fpv_pred, proj_weight):
    """Exact numpy replica of the reference CAM target (stop-gradient)."""
    xf = np.asarray(x, np.float32).reshape(B, C, NPOS)
    top_cls = np.argmax(np.asarray(x_fpv_pred, np.float32), axis=1)
    rows = np.asarray(proj_weight, np.float32)[top_cls]        # [B, C]
    cam = np.einsum('bc,bcn->bn', rows, xf)                    # [B, NPOS]
    cmin = cam.min(axis=1, keepdims=True)
    cmax = cam.max(axis=1, keepdims=True)
    cam_n = (cam - cmin) / (cmax - cmin)
    y = np.zeros_like(cam_n)
    # top-392 of 784 by value (ties measure-zero for random floats)
    idx = np.argpartition(-cam_n, N_TOKEN - 1, axis=1)[:, :N_TOKEN]
    np.put_along_axis(y, idx, np.take_along_axis(cam_n, idx, axis=1),
                      axis=1)
    return y


def run(inputs, trace=False):
    """Build (cached), run on 8 cores, return (loss, BassKernelResults)."""
    from concourse.bass_utils import run_bass_kernel_spmd

    if "nc" not in _cache:
        _cache["nc"] = _build_nc()
    nc = _cache["nc"]
    in_maps = _prep_in_maps(**inputs)
    res = run_bass_kernel_spmd(nc, in_maps, core_ids=list(range(B)),
                               trace=trace)
    y = _host_cam_gt(inputs["x"], inputs["x_fpv_pred"],
                     inputs["proj_weight"])
    sb = float(np.asarray(inputs["score_b"], np.float32).reshape(-1)[0])
    logits = np.stack([
        np.asarray(res.results[b]["out"], np.float32).reshape(NPOS)
        for b in range(B)]) + sb                               # [B, 784]
    xv = logits.astype(np.float64)
    yv = y.astype(np.float64)
    bce = (1.0 - yv) * xv + np.logaddexp(0.0, -xv)
    loss = np.float32(bce.mean())
    return loss, res


def kernel(**inputs):
    loss, _ = run(inputs, trace=False)
    return loss
